# revision 2
# baseline (speedup 1.0000x reference)
"""GQA attention layer (B=2, S=2048, D=4096, 32 Q heads / 8 KV heads, HD=128)
with rotary embeddings, causal mask, and output projection, on 8 trn2 cores.

Sharding: tensor-parallel over heads for QKV+attention (core c owns Q heads
[4c,4c+4) and KV head c), one AllToAll to re-shard the attention output from
head-sharded to token-sharded, then token-sharded output projection with the
full wo (no reduction needed). Host gathers the 8 token shards.

All matmuls run as float32r (full PE rate at free-dim>=256, ~1.5e-4 rel err).
Softmax is computed in transposed [k, q] layout so that:
  - scoresT = kT.T @ qT comes straight from the projection layout,
  - exp runs on ACT with the 1/sqrt(128) scale and a global stability
    constant folded into the activation,
  - denominators come from ones-vector matmuls on the PE,
  - y^T = v.T @ attnT needs no transposes (v is transposed once in phase 1).
RoPE in [hd, tok] layout pairs adjacent partitions; the pair swap runs as a
PE matmul with a signed permutation matrix, the cos/sin scaling on DVE.
"""
import sys

sys.path.insert(0, "/opt/trn_rl_repo")

import numpy as np

B, S, D = 2, 2048, 4096
NH, NL, HD = 32, 8, 128
CORES = 8
QH = NH // CORES          # 4 q heads per core
TOK = B * S               # 4096
TPC = TOK // CORES        # 512 tokens per core (output sharding)
NT = 256                  # phase-1 token block width
KB_D = D // 128           # 32 contraction blocks over D
QT_W = 512                # phase-2 q tile width
N_QT = S // QT_W          # 4 q tiles per batch
N_KB = S // 128           # 16 k blocks per batch
WO_NT = 256               # phase-4 dout block width
SCALE = 1.0 / np.sqrt(np.float32(HD))

_CACHE = {}


def _build_nc(mode, c_sub):
    """mode: 'causal' | 'full' | 'generic'. c_sub: global softmax shift."""
    import concourse.bacc as bacc
    import concourse.mybir as mybir
    import concourse.tile as tile
    from contextlib import ExitStack

    F32 = mybir.dt.float32
    F32R = mybir.dt.float32r
    AT = mybir.ActivationFunctionType
    OP = mybir.AluOpType

    nc = bacc.Bacc("TRN2", target_bir_lowering=False, debug=False,
                   num_devices=CORES)

    xT_d = nc.dram_tensor("xT", (D, TOK), F32, kind="ExternalInput").ap()
    wqkvT_d = nc.dram_tensor("wqkvT", (D, (QH + 2) * HD), F32,
                             kind="ExternalInput").ap()
    woT_d = nc.dram_tensor("woT", (D, D), F32, kind="ExternalInput").ap()
    cosP_d = nc.dram_tensor("cosP", (HD, TOK), F32, kind="ExternalInput").ap()
    sinP_d = nc.dram_tensor("sinP", (HD, TOK), F32, kind="ExternalInput").ap()
    if mode == "generic":
        biasT_d = nc.dram_tensor("biasT", (S, S), F32, kind="ExternalInput").ap()
    out_d = nc.dram_tensor("out", (TPC, D), F32, kind="ExternalOutput").ap()

    ident_h = nc.inline_tensor(np.eye(128, dtype=np.float32), name="ident")
    pswap = np.zeros((128, 128), dtype=np.float32)
    for i in range(64):
        pswap[2 * i, 2 * i + 1] = -1.0
        pswap[2 * i + 1, 2 * i] = 1.0
    pswapT_h = nc.inline_tensor(np.ascontiguousarray(pswap.T), name="pswapT")
    tri = np.zeros((4, 128, QT_W), dtype=np.float32)
    for j in range(4):
        for p in range(128):
            tri[j, p, p + 128 * j:] = 1.0
    tri_h = nc.inline_tensor(tri, name="trimask")
    ones_col_h = nc.inline_tensor(np.ones((128, 1), np.float32), name="ones_col")
    ones_row_h = nc.inline_tensor(np.ones((1, 128), np.float32), name="ones_row")

    NROW = (QH + 2) * HD          # 768 qkv rows per core
    NM = NROW // 128              # 6 m tiles (0..3 q heads, 4 kT, 5 vT)
    NNT = TOK // NT               # 16 phase-1 token blocks

    with tile.TileContext(nc) as tc, ExitStack() as glob:
        dram = glob.enter_context(tc.tile_pool(name="dram", bufs=1, space="DRAM"))
        consts = glob.enter_context(tc.tile_pool(name="consts", bufs=1))

        qkvT_t = dram.tile([NROW, TOK], F32R)
        v_t = dram.tile([TOK, HD], F32R)
        a2a_in_t = dram.tile([TOK, TPC], F32)
        a2a_out_t = dram.tile([TOK, TPC], F32)

        ident_sb = consts.tile([128, 128], F32)
        nc.sync.dma_start(ident_sb[:], ident_h.ap())
        pswapT_sb = consts.tile([128, 128], F32R)
        nc.sync.dma_start(pswapT_sb[:], pswapT_h.ap().bitcast(F32R))
        ones_col_sb = consts.tile([128, 1], F32R)
        nc.sync.dma_start(ones_col_sb[:], ones_col_h.ap().bitcast(F32R))
        ones_row_sb = consts.tile([1, 128], F32R)
        nc.sync.dma_start(ones_row_sb[:], ones_row_h.ap().bitcast(F32R))
        tri_sb = consts.tile([128, 4 * QT_W], F32)
        for j in range(4):
            nc.sync.dma_start(tri_sb[:, QT_W * j:QT_W * (j + 1)], tri_h.ap()[j])

        # ================= phase 1: qkv projection + rope + v transpose
        with ExitStack() as ctx1:
            p1_w = ctx1.enter_context(tc.tile_pool(name="p1_w", bufs=1))
            p1_x = ctx1.enter_context(tc.tile_pool(name="p1_x", bufs=2))
            p1_cs = ctx1.enter_context(tc.tile_pool(name="p1_cs", bufs=2))
            p1_st = ctx1.enter_context(tc.tile_pool(name="p1_st", bufs=3))
            p1_pa = ctx1.enter_context(tc.tile_pool(name="p1_pa", bufs=3, space="PSUM"))
            p1_pb = ctx1.enter_context(tc.tile_pool(name="p1_pb", bufs=2, space="PSUM"))
            p1_pt = ctx1.enter_context(tc.tile_pool(name="p1_pt", bufs=2, space="PSUM"))

            w_sb = []
            for kb in range(KB_D):
                wt = p1_w.tile([128, NROW], F32R, name=f"w{kb}")
                nc.sync.dma_start(wt[:], wqkvT_d[128 * kb:128 * (kb + 1), :].bitcast(F32R))
                w_sb.append(wt)

            for nt in range(NNT):
                c0 = NT * nt
                x_sb = []
                for kb in range(KB_D):
                    xt = p1_x.tile([128, NT], F32R, name=f"x{kb}")
                    nc.sync.dma_start(
                        xt[:], xT_d[128 * kb:128 * (kb + 1), c0:c0 + NT].bitcast(F32R))
                    x_sb.append(xt)
                cos_sb = p1_cs.tile([128, NT], F32, name="cos_sb")
                nc.sync.dma_start(cos_sb[:], cosP_d[:, c0:c0 + NT])
                sin_sb = p1_cs.tile([128, NT], F32, name="sin_sb")
                nc.sync.dma_start(sin_sb[:], sinP_d[:, c0:c0 + NT])

                for m in range(NM):
                    pa = p1_pa.tile([128, NT], F32, name="pa")
                    for kb in range(KB_D):
                        nc.tensor.matmul(pa[:], w_sb[kb][:, 128 * m:128 * (m + 1)],
                                         x_sb[kb][:],
                                         start=(kb == 0), stop=(kb == KB_D - 1))
                    if m < NM - 1:
                        # rope for q heads + k head
                        a_sb = p1_st.tile([128, NT], F32R, name="a_sb")
                        nc.vector.tensor_copy(a_sb[:], pa[:])
                        pb = p1_pb.tile([128, NT], F32, name="pb")
                        nc.tensor.matmul(pb[:], pswapT_sb[:], a_sb[:],
                                         start=True, stop=True)
                        tcos = p1_st.tile([128, NT], F32, name="tcos")
                        nc.vector.tensor_tensor(tcos[:], pa[:], cos_sb[:], op=OP.mult)
                        tsin = p1_st.tile([128, NT], F32, name="tsin")
                        nc.vector.tensor_tensor(tsin[:], pb[:], sin_sb[:], op=OP.mult)
                        roped = p1_st.tile([128, NT], F32R, name="roped")
                        nc.vector.tensor_tensor(roped[:], tcos[:], tsin[:], op=OP.add)
                        nc.sync.dma_start(
                            qkvT_t[128 * m:128 * (m + 1), c0:c0 + NT], roped[:])
                    else:
                        # vT -> v natural via PE transpose
                        vst = p1_st.tile([128, NT], F32, name="vst")
                        nc.scalar.copy(vst[:], pa[:])
                        for j in range(NT // 128):
                            pt = p1_pt.tile([128, 128], F32, name="pt")
                            nc.tensor.transpose(
                                pt[:], vst[:, 128 * j:128 * (j + 1)], ident_sb[:])
                            trv = p1_st.tile([128, 128], F32R, name="trv")
                            nc.vector.tensor_copy(trv[:], pt[:])
                            nc.sync.dma_start(
                                v_t[c0 + 128 * j:c0 + 128 * (j + 1), :], trv[:])

        # ================= phase 4 woT pool opened early for prefetch
        p4_w = glob.enter_context(tc.tile_pool(name="p4_w", bufs=3))

        # ================= phase 2: attention per (batch, head)
        with ExitStack() as ctx2:
            p2_qk = ctx2.enter_context(tc.tile_pool(name="p2_qk", bufs=2))
            p2_v = ctx2.enter_context(tc.tile_pool(name="p2_v", bufs=2))
            p2_at = ctx2.enter_context(tc.tile_pool(name="p2_at", bufs=4))
            p2_ms = ctx2.enter_context(tc.tile_pool(name="p2_ms", bufs=3))
            p2_psc = ctx2.enter_context(tc.tile_pool(name="p2_psc", bufs=2, space="PSUM"))
            p2_py = ctx2.enter_context(tc.tile_pool(name="p2_py", bufs=2, space="PSUM"))
            p2_ps = ctx2.enter_context(tc.tile_pool(name="p2_ps", bufs=2, space="PSUM"))
            p2_pr = ctx2.enter_context(tc.tile_pool(name="p2_pr", bufs=2, space="PSUM"))
            if mode == "generic":
                p2_bias = ctx2.enter_context(tc.tile_pool(name="p2_bias", bufs=4))

            for b in range(B):
                t0 = S * b
                kT = p2_qk.tile([128, S], F32R, name="kT")
                nc.sync.dma_start(kT[:], qkvT_t[QH * 128:(QH + 1) * 128, t0:t0 + S])
                v_sb = []
                for i in range(N_KB):
                    vt = p2_v.tile([128, HD], F32R, name=f"v{i}")
                    nc.sync.dma_start(
                        vt[:], v_t[t0 + 128 * i:t0 + 128 * (i + 1), :])
                    v_sb.append(vt)
                for h in range(QH):
                    qT = p2_qk.tile([128, S], F32R, name="qT")
                    nc.sync.dma_start(qT[:], qkvT_t[128 * h:128 * (h + 1), t0:t0 + S])
                    for qt in range(N_QT):
                        kb_max = 4 * qt + 4 if mode == "causal" else N_KB
                        py = p2_py.tile([128, QT_W], F32, name="py")
                        ps_s = p2_ps.tile([1, QT_W], F32, name="ps_s")
                        for kb in range(kb_max):
                            psc = p2_psc.tile([128, QT_W], F32, name="psc")
                            nc.tensor.matmul(psc[:], kT[:, 128 * kb:128 * (kb + 1)],
                                             qT[:, QT_W * qt:QT_W * (qt + 1)],
                                             start=True, stop=True)
                            if mode == "generic":
                                bt = p2_bias.tile([128, QT_W], F32, name="bt")
                                nc.sync.dma_start(
                                    bt[:], biasT_d[128 * kb:128 * (kb + 1),
                                                   QT_W * qt:QT_W * (qt + 1)])
                                nc.vector.tensor_tensor(psc[:], psc[:], bt[:], op=OP.add)
                            at = p2_at.tile([128, QT_W], F32R, name="at")
                            nc.scalar.activation(at[:], psc[:], AT.Exp,
                                                 bias=-float(c_sub), scale=float(SCALE))
                            if mode == "causal" and kb >= 4 * qt:
                                j = kb - 4 * qt
                                nc.vector.tensor_tensor(
                                    at[:], at[:].bitcast(F32),
                                    tri_sb[:, QT_W * j:QT_W * (j + 1)], op=OP.mult)
                            nc.tensor.matmul(py[:], v_sb[kb][:], at[:],
                                             start=(kb == 0), stop=(kb == kb_max - 1))
                            nc.tensor.matmul(ps_s[:], ones_col_sb[:], at[:],
                                             start=(kb == 0), stop=(kb == kb_max - 1))
                        # normalize: yT_sb = py * (1/sums) replicated over partitions
                        s_eps = p2_ms.tile([1, QT_W], F32, name="s_eps")
                        nc.vector.tensor_scalar_add(s_eps[:], ps_s[:], 1e-30)
                        recip = p2_ms.tile([1, QT_W], F32R, name="recip")
                        with nc.allow_low_precision(reason="f32r recip, 1e-4 ok"):
                            nc.vector.reciprocal(recip[:], s_eps[:])
                        pr = p2_pr.tile([128, QT_W], F32, name="pr")
                        nc.tensor.matmul(pr[:], ones_row_sb[:], recip[:],
                                         start=True, stop=True)
                        rep_sb = p2_ms.tile([128, QT_W], F32, name="rep_sb")
                        nc.scalar.copy(rep_sb[:], pr[:])
                        yT_sb = p2_ms.tile([128, QT_W], F32, name="yT_sb")
                        nc.vector.tensor_tensor(yT_sb[:], py[:], rep_sb[:], op=OP.mult)
                        # A2A input: chunk j = 4b + qt, rows 128h..
                        j = 4 * b + qt
                        nc.sync.dma_start(
                            a2a_in_t[TPC * j + 128 * h:TPC * j + 128 * (h + 1), :],
                            yT_sb[:])

        # ================= phase 3: AllToAll (head-shard -> token-shard)
        nc.gpsimd.collective_compute(
            "AllToAll", mybir.AluOpType.bypass,
            replica_groups=[list(range(CORES))],
            ins=[a2a_in_t[:]], outs=[a2a_out_t[:]],
        )

        # ================= phase 4: output projection out = y @ wo.T
        with ExitStack() as ctx4:
            p4_y = ctx4.enter_context(tc.tile_pool(name="p4_y", bufs=1))
            p4_st = ctx4.enter_context(tc.tile_pool(name="p4_st", bufs=4))
            p4_po = ctx4.enter_context(tc.tile_pool(name="p4_po", bufs=4, space="PSUM"))

            y_sb = []
            for kb in range(KB_D):
                yt = p4_y.tile([128, TPC], F32R, name=f"y{kb}")
                nc.sync.dma_start(
                    yt[:], a2a_out_t[128 * kb:128 * (kb + 1), :].bitcast(F32R))
                y_sb.append(yt)
            for do in range(D // WO_NT):
                wo_sb = []
                for kb in range(KB_D):
                    wt = p4_w.tile([128, WO_NT], F32R, name=f"wo{kb}")
                    nc.sync.dma_start(
                        wt[:], woT_d[128 * kb:128 * (kb + 1),
                                     WO_NT * do:WO_NT * (do + 1)].bitcast(F32R))
                    wo_sb.append(wt)
                for tb in range(TPC // 128):
                    po = p4_po.tile([128, WO_NT], F32, name="po")
                    for kb in range(KB_D):
                        nc.tensor.matmul(po[:], y_sb[kb][:, 128 * tb:128 * (tb + 1)],
                                         wo_sb[kb][:],
                                         start=(kb == 0), stop=(kb == KB_D - 1))
                    o_sb = p4_st.tile([128, WO_NT], F32, name="o_sb")
                    nc.scalar.copy(o_sb[:], po[:])
                    nc.sync.dma_start(
                        out_d[128 * tb:128 * (tb + 1), WO_NT * do:WO_NT * (do + 1)],
                        o_sb[:])

    nc.compile()
    return nc


def _prepare(x, freqs_cis, mask, wqkv_w, wo_w):
    """Host-side prep: mode detection, stability constant, input maps."""
    x = np.asarray(x, dtype=np.float32)
    freqs_cis = np.asarray(freqs_cis, dtype=np.float32)
    mask = np.asarray(mask)
    wqkv_w = np.asarray(wqkv_w, dtype=np.float32)
    wo_w = np.asarray(wo_w, dtype=np.float32)

    m2 = mask.reshape(mask.shape[-2], mask.shape[-1])
    if np.array_equal(m2, np.tril(np.ones((S, S), dtype=bool))):
        mode = "causal"
    elif m2.all():
        mode = "full"
    else:
        mode = "generic"

    x2 = x.reshape(TOK, D)
    xT = np.ascontiguousarray(x2.T)
    woT = np.ascontiguousarray(wo_w.T)

    cos = freqs_cis[:, :, 0].T          # [64, S]
    sin = freqs_cis[:, :, 1].T
    cosP = np.repeat(cos, 2, axis=0)    # [128, S]
    sinP = np.repeat(sin, 2, axis=0)
    cosP = np.ascontiguousarray(np.tile(cosP, (1, B)))  # [128, TOK]
    sinP = np.ascontiguousarray(np.tile(sinP, (1, B)))

    # softmax stability probe: rope'd scores for head 0, batch 0, 128 q rows
    wq0 = wqkv_w[:HD]                   # [128, D]
    wk0 = wqkv_w[NH * HD:NH * HD + HD]  # [128, D]
    qs = x2[:128] @ wq0.T               # [128, 128]
    ks = x2[:S] @ wk0.T                 # [S, 128]

    def rope_np(t, fc):
        ts = t.reshape(t.shape[0], HD // 2, 2)
        c, s_ = fc[:t.shape[0], :, 0], fc[:t.shape[0], :, 1]
        out = np.empty_like(ts)
        out[:, :, 0] = ts[:, :, 0] * c - ts[:, :, 1] * s_
        out[:, :, 1] = ts[:, :, 1] * c + ts[:, :, 0] * s_
        return out.reshape(t.shape)

    qs = rope_np(qs, freqs_cis)
    ks = rope_np(ks, freqs_cis)
    smax = float(np.max(np.abs(qs @ ks.T)) * SCALE)
    c_sub = 0.0 if smax < 25.0 else smax + 5.0

    in_maps = []
    for c in range(CORES):
        wq_c = wqkv_w[QH * HD * c:QH * HD * (c + 1)]
        wk_c = wqkv_w[NH * HD + HD * c:NH * HD + HD * (c + 1)]
        wv_c = wqkv_w[(NH + NL) * HD + HD * c:(NH + NL) * HD + HD * (c + 1)]
        wqkvT_c = np.ascontiguousarray(np.vstack([wq_c, wk_c, wv_c]).T)
        m = {"xT": xT, "wqkvT": wqkvT_c, "woT": woT, "cosP": cosP, "sinP": sinP}
        if mode == "generic":
            m["biasT"] = np.ascontiguousarray(
                np.where(m2.T, np.float32(0), np.float32(-1e30)))
        in_maps.append(m)
    return mode, c_sub, in_maps


def _get_nc(mode, c_sub):
    key = (mode, round(float(c_sub), 3))
    if key not in _CACHE:
        _CACHE[key] = _build_nc(mode, c_sub)
    return _CACHE[key]


def kernel(x, freqs_cis, mask, wqkv_w, wo_w):
    from concourse import bass_utils
    mode, c_sub, in_maps = _prepare(x, freqs_cis, mask, wqkv_w, wo_w)
    nc = _get_nc(mode, c_sub)
    res = bass_utils.run_bass_kernel_spmd(nc, in_maps, core_ids=list(range(CORES)))
    out = np.concatenate([res.results[c]["out"] for c in range(CORES)], axis=0)
    return out.reshape(B, S, D)


# revision 8
# speedup vs baseline: 1.2943x; 1.2943x over previous
"""GQA attention layer (B=2, S=2048, D=4096, 32 Q heads / 8 KV heads, HD=128)
with rotary embeddings, causal mask, and output projection, on 8 trn2 cores.

Sharding: tensor-parallel over heads for QKV+attention (core c owns Q heads
[4c,4c+4) and KV head c), two AllToAlls (split by head-pair, overlapped with
compute) to re-shard the attention output from head-sharded to token-sharded,
then token-sharded output projection with the full wo (no reduction needed).
Host gathers the 8 token shards.

All matmuls run as float32r (full PE rate at free-dim>=256, ~1.5e-4 rel err).
Softmax is computed in transposed [k, q] layout so that:
  - scoresT = kT.T @ qT comes straight from the projection layout,
  - exp runs on ACT with the 1/sqrt(128) scale and a global stability
    constant folded into the activation,
  - denominators come from ones-vector matmuls on the PE,
  - y^T = v.T @ attnT needs no transposes (v is transposed once in phase 1).
RoPE in [hd, tok] layout pairs adjacent partitions; the pair swap runs as a
PE matmul with a signed permutation matrix, the cos/sin scaling on DVE.
"""
import sys

sys.path.insert(0, "/opt/trn_rl_repo")

import numpy as np

B, S, D = 2, 2048, 4096
NH, NL, HD = 32, 8, 128
CORES = 8
QH = NH // CORES          # 4 q heads per core
TOK = B * S               # 4096
TPC = TOK // CORES        # 512 tokens per core (output sharding)
NT = 256                  # phase-1 token block width
KB_D = D // 128           # 32 contraction blocks over D
QT_W = 512                # phase-2 q tile width
N_QT = S // QT_W          # 4 q tiles per batch
N_KB = S // 128           # 16 k blocks per batch
WO_NT = 256               # phase-4 dout block width
SCALE = 1.0 / np.sqrt(np.float32(HD))

_CACHE = {}


def _build_nc(mode, c_sub, sim=False):
    """mode: 'causal' | 'full' | 'generic'. c_sub: global softmax shift.
    sim=True: single-core TimelineSim variant (collective replaced by DMAs)."""
    import concourse.bacc as bacc
    import concourse.mybir as mybir
    import concourse.tile as tile
    from contextlib import ExitStack

    F32 = mybir.dt.float32
    F32R = mybir.dt.float32r
    AT = mybir.ActivationFunctionType
    OP = mybir.AluOpType

    nc = bacc.Bacc("TRN2", target_bir_lowering=False, debug=False,
                   num_devices=1 if sim else CORES)

    xT_d = nc.dram_tensor("xT", (D, TOK), F32, kind="ExternalInput").ap()
    wqkvT_d = nc.dram_tensor("wqkvT", (D, (QH + 2) * HD), F32,
                             kind="ExternalInput").ap()
    woT_d = nc.dram_tensor("woT", (D, D), F32, kind="ExternalInput").ap()
    cosP_d = nc.dram_tensor("cosP", (HD, TOK), F32, kind="ExternalInput").ap()
    sinP_d = nc.dram_tensor("sinP", (HD, TOK), F32, kind="ExternalInput").ap()
    if mode == "generic":
        biasT_d = nc.dram_tensor("biasT", (S, S), F32, kind="ExternalInput").ap()
    out_d = nc.dram_tensor("out", (TPC, D), F32, kind="ExternalOutput").ap()

    ident_h = nc.inline_tensor(np.eye(128, dtype=np.float32), name="ident")
    pswap = np.zeros((128, 128), dtype=np.float32)
    for i in range(64):
        pswap[2 * i, 2 * i + 1] = -1.0
        pswap[2 * i + 1, 2 * i] = 1.0
    pswapT_h = nc.inline_tensor(np.ascontiguousarray(pswap.T), name="pswapT")
    tri = np.zeros((4, 128, QT_W), dtype=np.float32)
    for j in range(4):
        for p in range(128):
            tri[j, p, p + 128 * j:] = 1.0
    tri_h = nc.inline_tensor(tri, name="trimask")
    ones_col_h = nc.inline_tensor(np.ones((128, 1), np.float32), name="ones_col")
    ones_row_h = nc.inline_tensor(np.ones((1, 128), np.float32), name="ones_row")

    NROW = (QH + 2) * HD          # 768 qkv rows per core
    NM = NROW // 128              # 6 m tiles (0..3 q heads, 4 kT, 5 vT)
    NNT = TOK // NT               # 16 phase-1 token blocks
    NPAIR = KB_D // 2             # 16 kb pairs

    with tile.TileContext(nc) as tc, ExitStack() as glob:
        dram = glob.enter_context(tc.tile_pool(name="dram", bufs=1, space="DRAM"))
        consts = glob.enter_context(tc.tile_pool(name="consts", bufs=1))

        qkvT_t = dram.tile([NROW, TOK], F32R)
        v_t = dram.tile([TOK, HD], F32R)
        # split A2A: hp=0 carries head-locals {0,1}, hp=1 carries {2,3}
        a2a_in = [dram.tile([TOK // 2, TPC], F32, name=f"a2a_in{hp}")
                  for hp in range(2)]
        a2a_out = [dram.tile([TOK // 2, TPC], F32, name=f"a2a_out{hp}")
                   for hp in range(2)]

        ident_sb = consts.tile([128, 128], F32)
        nc.sync.dma_start(ident_sb[:], ident_h.ap())
        pswapT_sb = consts.tile([128, 128], F32R)
        nc.sync.dma_start(pswapT_sb[:], pswapT_h.ap().bitcast(F32R))
        ones_col_sb = consts.tile([128, 1], F32R)
        nc.sync.dma_start(ones_col_sb[:], ones_col_h.ap().bitcast(F32R))
        ones_row_sb = consts.tile([1, 128], F32R)
        nc.sync.dma_start(ones_row_sb[:], ones_row_h.ap().bitcast(F32R))
        tri_sb = consts.tile([128, 4 * QT_W], F32)
        for j in range(4):
            nc.sync.dma_start(tri_sb[:, QT_W * j:QT_W * (j + 1)], tri_h.ap()[j])

        # ================= phase 1: qkv projection + rope + v transpose
        with ExitStack() as ctx1:
            p1_w = ctx1.enter_context(tc.tile_pool(name="p1_w", bufs=1))
            p1_x = ctx1.enter_context(tc.tile_pool(name="p1_x", bufs=2))
            p1_cs = ctx1.enter_context(tc.tile_pool(name="p1_cs", bufs=2))
            p1_st = ctx1.enter_context(tc.tile_pool(name="p1_st", bufs=3))
            p1_so = ctx1.enter_context(tc.tile_pool(name="p1_so", bufs=2))
            p1_pa = ctx1.enter_context(tc.tile_pool(name="p1_pa", bufs=3, space="PSUM"))
            p1_pb = ctx1.enter_context(tc.tile_pool(name="p1_pb", bufs=2, space="PSUM"))
            p1_pt = ctx1.enter_context(tc.tile_pool(name="p1_pt", bufs=2, space="PSUM"))

            # kb-paired weight tiles; interleave with nt=0 x DMAs for startup
            w_sb = []
            x0_sb = []
            for i in range(NPAIR):
                wt = p1_w.tile([128, 2 * NROW], F32R, name=f"w{i}")
                nc.sync.dma_start(
                    wt[:].rearrange("p (two c) -> p two c", two=2),
                    wqkvT_d[256 * i:256 * (i + 1), :]
                    .rearrange("(two p) c -> p two c", p=128).bitcast(F32R))
                w_sb.append(wt)
                xt = p1_x.tile([128, 2 * NT], F32R, name=f"x{i}")
                nc.sync.dma_start(
                    xt[:].rearrange("p (two c) -> p two c", two=2),
                    xT_d[256 * i:256 * (i + 1), 0:NT]
                    .rearrange("(two p) c -> p two c", p=128).bitcast(F32R))
                x0_sb.append(xt)

            def wv(kb):       # lhsT view of wqkv kb-block
                return w_sb[kb // 2][:, NROW * (kb % 2):NROW * (kb % 2) + NROW]

            for nt in range(NNT):
                c0 = NT * nt
                if nt == 0:
                    x_sb = x0_sb
                else:
                    x_sb = []
                    for i in range(NPAIR):
                        xt = p1_x.tile([128, 2 * NT], F32R, name=f"x{i}")
                        nc.sync.dma_start(
                            xt[:].rearrange("p (two c) -> p two c", two=2),
                            xT_d[256 * i:256 * (i + 1), c0:c0 + NT]
                            .rearrange("(two p) c -> p two c", p=128).bitcast(F32R))
                        x_sb.append(xt)

                def xv(kb):
                    return x_sb[kb // 2][:, NT * (kb % 2):NT * (kb % 2) + NT]

                cos_sb = p1_cs.tile([128, NT], F32, name="cos_sb")
                nc.sync.dma_start(cos_sb[:], cosP_d[:, c0:c0 + NT])
                sin_sb = p1_cs.tile([128, NT], F32, name="sin_sb")
                nc.sync.dma_start(sin_sb[:], sinP_d[:, c0:c0 + NT])

                stage = p1_so.tile([128, (NM - 1) * NT], F32R, name="stage")
                for m in range(NM):
                    pa = p1_pa.tile([128, NT], F32, name="pa")
                    for kb in range(KB_D):
                        nc.tensor.matmul(pa[:], wv(kb)[:, 128 * m:128 * (m + 1)],
                                         xv(kb),
                                         start=(kb == 0), stop=(kb == KB_D - 1))
                    if m < NM - 1:
                        # rope for q heads + k head; write into packed stage
                        a_sb = p1_st.tile([128, NT], F32R, name="a_sb")
                        nc.vector.tensor_copy(a_sb[:], pa[:])
                        pb = p1_pb.tile([128, NT], F32, name="pb")
                        nc.tensor.matmul(pb[:], pswapT_sb[:], a_sb[:],
                                         start=True, stop=True)
                        tcos = p1_st.tile([128, NT], F32, name="tcos")
                        nc.vector.tensor_tensor(tcos[:], pa[:], cos_sb[:], op=OP.mult)
                        tsin = p1_st.tile([128, NT], F32, name="tsin")
                        nc.vector.tensor_tensor(tsin[:], pb[:], sin_sb[:], op=OP.mult)
                        nc.vector.tensor_tensor(stage[:, NT * m:NT * (m + 1)],
                                                tcos[:], tsin[:], op=OP.add)
                    else:
                        # vT -> v natural via PE transpose, packed write-out
                        vst = p1_st.tile([128, NT], F32, name="vst")
                        nc.scalar.copy(vst[:], pa[:])
                        vpack = p1_st.tile([128, NT], F32R, name="vpack")
                        for j in range(NT // 128):
                            pt = p1_pt.tile([128, 128], F32, name="pt")
                            nc.tensor.transpose(
                                pt[:], vst[:, 128 * j:128 * (j + 1)], ident_sb[:])
                            nc.vector.tensor_copy(
                                vpack[:, 128 * j:128 * (j + 1)], pt[:])
                        nc.sync.dma_start(
                            v_t[c0:c0 + NT, :]
                            .rearrange("(j p) d -> p j d", p=128),
                            vpack[:].rearrange("p (j d) -> p j d", d=128))
                # one packed DMA for the 5 roped m-tiles
                nc.sync.dma_start(
                    qkvT_t[0:(NM - 1) * 128, c0:c0 + NT]
                    .rearrange("(m p) c -> p m c", p=128),
                    stage[:].rearrange("p (m c) -> p m c", c=NT))

        # ================= phase 4 woT pool opened early for prefetch
        p4_w = glob.enter_context(tc.tile_pool(name="p4_w", bufs=2))

        # ================= phase 2: attention, head-pair outer for split A2A
        with ExitStack() as ctx2:
            p2_qk = ctx2.enter_context(tc.tile_pool(name="p2_qk", bufs=2))
            p2_v = ctx2.enter_context(tc.tile_pool(name="p2_v", bufs=2))
            p2_at = ctx2.enter_context(tc.tile_pool(name="p2_at", bufs=4))
            p2_ms = ctx2.enter_context(tc.tile_pool(name="p2_ms", bufs=3))
            p2_psc = ctx2.enter_context(tc.tile_pool(name="p2_psc", bufs=3, space="PSUM"))
            p2_py = ctx2.enter_context(tc.tile_pool(name="p2_py", bufs=2, space="PSUM"))
            p2_ps = ctx2.enter_context(tc.tile_pool(name="p2_ps", bufs=2, space="PSUM"))
            p2_pr = ctx2.enter_context(tc.tile_pool(name="p2_pr", bufs=1, space="PSUM"))
            if mode == "generic":
                p2_bias = ctx2.enter_context(tc.tile_pool(name="p2_bias", bufs=4))

            for hp in range(2):
                for b in range(B):
                    t0 = S * b
                    kT = p2_qk.tile([128, S], F32R, name="kT")
                    nc.sync.dma_start(kT[:],
                                      qkvT_t[QH * 128:(QH + 1) * 128, t0:t0 + S])
                    v_sb = []
                    for i in range(N_KB // 4):
                        vt = p2_v.tile([128, 4 * HD], F32R, name=f"v{i}")
                        nc.sync.dma_start(
                            vt[:].rearrange("p (j d) -> p j d", d=HD),
                            v_t[t0 + 512 * i:t0 + 512 * (i + 1), :]
                            .rearrange("(j p) d -> p j d", p=128))
                        v_sb.append(vt)

                    def vvw(kb):
                        return v_sb[kb // 4][:, HD * (kb % 4):HD * (kb % 4) + HD]

                    for h in (2 * hp, 2 * hp + 1):
                        qT = p2_qk.tile([128, S], F32R, name="qT")
                        nc.sync.dma_start(
                            qT[:], qkvT_t[128 * h:128 * (h + 1), t0:t0 + S])
                        for qt in range(N_QT):
                            kb_max = 4 * qt + 4 if mode == "causal" else N_KB
                            py = p2_py.tile([128, QT_W], F32, name="py")
                            s_t = p2_ps.tile([1, QT_W], F32, name="s_t")
                            ps_s = s_t[:]
                            for kb in range(kb_max):
                                psc = p2_psc.tile([128, QT_W], F32, name="psc")
                                nc.tensor.matmul(
                                    psc[:], kT[:, 128 * kb:128 * (kb + 1)],
                                    qT[:, QT_W * qt:QT_W * (qt + 1)],
                                    start=True, stop=True)
                                if mode == "generic":
                                    bt = p2_bias.tile([128, QT_W], F32, name="bt")
                                    nc.sync.dma_start(
                                        bt[:], biasT_d[128 * kb:128 * (kb + 1),
                                                       QT_W * qt:QT_W * (qt + 1)])
                                    nc.vector.tensor_tensor(psc[:], psc[:], bt[:],
                                                            op=OP.add)
                                at = p2_at.tile([128, QT_W], F32R, name="at")
                                nc.scalar.activation(at[:], psc[:], AT.Exp,
                                                     bias=-float(c_sub),
                                                     scale=float(SCALE))
                                if mode == "causal" and kb >= 4 * qt:
                                    j = kb - 4 * qt
                                    w = 128 * j + 128
                                    nc.vector.tensor_tensor(
                                        at[:, 0:w], at[:, 0:w].bitcast(F32),
                                        tri_sb[:, QT_W * j:QT_W * j + w],
                                        op=OP.mult)
                                nc.tensor.matmul(py[:], vvw(kb), at[:],
                                                 start=(kb == 0),
                                                 stop=(kb == kb_max - 1))
                                nc.tensor.matmul(ps_s, ones_col_sb[:], at[:],
                                                 start=(kb == 0),
                                                 stop=(kb == kb_max - 1),
                                                 skip_group_check=True)
                            # normalize: yT = py * (1/sums) replicated over parts
                            recip = p2_ms.tile([1, QT_W], F32R, name="recip")
                            with nc.allow_low_precision(reason="f32r recip"):
                                nc.vector.reciprocal(recip[:], ps_s)
                            pr = p2_pr.tile([128, QT_W], F32, name="pr")
                            nc.tensor.matmul(pr[:], ones_row_sb[:], recip[:],
                                             start=True, stop=True)
                            rep_sb = p2_ms.tile([128, QT_W], F32, name="rep_sb")
                            nc.scalar.copy(rep_sb[:], pr[:])
                            yT_sb = p2_ms.tile([128, QT_W], F32, name="yT_sb")
                            nc.vector.tensor_tensor(yT_sb[:], py[:], rep_sb[:],
                                                    op=OP.mult)
                            # A2A input: chunk j = 4b + qt, rows 128*(h%2)
                            j = 4 * b + qt
                            r0 = (TPC // 2) * j + 128 * (h % 2)
                            nc.sync.dma_start(
                                a2a_in[hp][r0:r0 + 128, :], yT_sb[:])
                # -------- A2A for this head-pair (overlaps later compute)
                if sim:
                    for j in range(CORES):
                        nc.sync.dma_start(
                            a2a_out[hp][(TPC // 2) * j:(TPC // 2) * (j + 1), :],
                            a2a_in[hp][(TPC // 2) * j:(TPC // 2) * (j + 1), :])
                else:
                    nc.gpsimd.collective_compute(
                        "AllToAll", mybir.AluOpType.bypass,
                        replica_groups=[list(range(CORES))],
                        ins=[a2a_in[hp][:]], outs=[a2a_out[hp][:]],
                    )

        # ================= phase 4: output projection out = y @ wo.T
        # yT_full row for global kb: i=kb//4, hl=kb%4 -> a2a_out[hl//2]
        # rows 256*i + 128*(hl%2). Emit hp=0 kbs first so phase 4 starts
        # while A2A #1 is the only one finished.
        kb_order = [kb for kb in range(KB_D) if (kb % 4) < 2] + \
                   [kb for kb in range(KB_D) if (kb % 4) >= 2]
        with ExitStack() as ctx4:
            p4_y = ctx4.enter_context(tc.tile_pool(name="p4_y", bufs=1))
            p4_st = ctx4.enter_context(tc.tile_pool(name="p4_st", bufs=4))
            p4_po = ctx4.enter_context(tc.tile_pool(name="p4_po", bufs=4, space="PSUM"))

            y_sb = {}
            for kb in kb_order:
                i, hl = kb // 4, kb % 4
                r0 = 256 * i + 128 * (hl % 2)
                yt = p4_y.tile([128, TPC], F32R, name=f"y{kb}")
                nc.sync.dma_start(
                    yt[:], a2a_out[hl // 2][r0:r0 + 128, :].bitcast(F32R))
                y_sb[kb] = yt
            for do in range(D // WO_NT):
                wo_sb = []
                for i in range(NPAIR):
                    wt = p4_w.tile([128, 2 * WO_NT], F32R, name=f"wo{i}")
                    nc.sync.dma_start(
                        wt[:].rearrange("p (two c) -> p two c", two=2),
                        woT_d[256 * i:256 * (i + 1),
                              WO_NT * do:WO_NT * (do + 1)]
                        .rearrange("(two p) c -> p two c", p=128).bitcast(F32R))
                    wo_sb.append(wt)

                def wov(kb):
                    return wo_sb[kb // 2][:, WO_NT * (kb % 2):WO_NT * (kb % 2 + 1)]

                for tb in range(TPC // 128):
                    po = p4_po.tile([128, WO_NT], F32, name="po")
                    for n, kb in enumerate(kb_order):
                        nc.tensor.matmul(po[:], y_sb[kb][:, 128 * tb:128 * (tb + 1)],
                                         wov(kb),
                                         start=(n == 0), stop=(n == KB_D - 1))
                    o_sb = p4_st.tile([128, WO_NT], F32, name="o_sb")
                    nc.scalar.copy(o_sb[:], po[:])
                    nc.sync.dma_start(
                        out_d[128 * tb:128 * (tb + 1), WO_NT * do:WO_NT * (do + 1)],
                        o_sb[:])

    nc.compile()
    return nc


def _prepare(x, freqs_cis, mask, wqkv_w, wo_w):
    """Host-side prep: mode detection, stability constant, input maps."""
    x = np.asarray(x, dtype=np.float32)
    freqs_cis = np.asarray(freqs_cis, dtype=np.float32)
    mask = np.asarray(mask)
    wqkv_w = np.asarray(wqkv_w, dtype=np.float32)
    wo_w = np.asarray(wo_w, dtype=np.float32)

    m2 = mask.reshape(mask.shape[-2], mask.shape[-1])
    if np.array_equal(m2, np.tril(np.ones((S, S), dtype=bool))):
        mode = "causal"
    elif m2.all():
        mode = "full"
    else:
        mode = "generic"

    x2 = x.reshape(TOK, D)
    xT = np.ascontiguousarray(x2.T)
    woT = np.ascontiguousarray(wo_w.T)

    cos = freqs_cis[:, :, 0].T          # [64, S]
    sin = freqs_cis[:, :, 1].T
    cosP = np.repeat(cos, 2, axis=0)    # [128, S]
    sinP = np.repeat(sin, 2, axis=0)
    cosP = np.ascontiguousarray(np.tile(cosP, (1, B)))  # [128, TOK]
    sinP = np.ascontiguousarray(np.tile(sinP, (1, B)))

    # softmax stability probe: rope'd scores for head 0, batch 0, 128 q rows
    wq0 = wqkv_w[:HD]                   # [128, D]
    wk0 = wqkv_w[NH * HD:NH * HD + HD]  # [128, D]
    qs = x2[:128] @ wq0.T               # [128, 128]
    ks = x2[:S] @ wk0.T                 # [S, 128]

    def rope_np(t, fc):
        ts = t.reshape(t.shape[0], HD // 2, 2)
        c, s_ = fc[:t.shape[0], :, 0], fc[:t.shape[0], :, 1]
        out = np.empty_like(ts)
        out[:, :, 0] = ts[:, :, 0] * c - ts[:, :, 1] * s_
        out[:, :, 1] = ts[:, :, 1] * c + ts[:, :, 0] * s_
        return out.reshape(t.shape)

    qs = rope_np(qs, freqs_cis)
    ks = rope_np(ks, freqs_cis)
    smax = float(np.max(np.abs(qs @ ks.T)) * SCALE)
    c_sub = 0.0 if smax < 25.0 else smax + 5.0

    in_maps = []
    for c in range(CORES):
        wq_c = wqkv_w[QH * HD * c:QH * HD * (c + 1)]
        wk_c = wqkv_w[NH * HD + HD * c:NH * HD + HD * (c + 1)]
        wv_c = wqkv_w[(NH + NL) * HD + HD * c:(NH + NL) * HD + HD * (c + 1)]
        wqkvT_c = np.ascontiguousarray(np.vstack([wq_c, wk_c, wv_c]).T)
        m = {"xT": xT, "wqkvT": wqkvT_c, "woT": woT, "cosP": cosP, "sinP": sinP}
        if mode == "generic":
            m["biasT"] = np.ascontiguousarray(
                np.where(m2.T, np.float32(0), np.float32(-1e30)))
        in_maps.append(m)
    return mode, c_sub, in_maps


def _get_nc(mode, c_sub):
    key = (mode, round(float(c_sub), 3))
    if key not in _CACHE:
        _CACHE[key] = _build_nc(mode, c_sub)
    return _CACHE[key]


def kernel(x, freqs_cis, mask, wqkv_w, wo_w):
    from concourse import bass_utils
    mode, c_sub, in_maps = _prepare(x, freqs_cis, mask, wqkv_w, wo_w)
    nc = _get_nc(mode, c_sub)
    res = bass_utils.run_bass_kernel_spmd(nc, in_maps, core_ids=list(range(CORES)))
    out = np.concatenate([res.results[c]["out"] for c in range(CORES)], axis=0)
    return out.reshape(B, S, D)


# revision 9
# speedup vs baseline: 76.0954x; 58.7941x over previous
"""GQA attention layer (B=2, S=2048, D=4096, 32 Q heads / 8 KV heads, HD=128)
with rotary embeddings, causal mask, and output projection, on 8 trn2 cores.

Sharding: tensor-parallel over heads for QKV+attention (core c owns Q heads
[4c,4c+4) and KV head c), two AllToAlls (split by head-pair, overlapped with
compute) to re-shard the attention output from head-sharded to token-sharded,
then token-sharded output projection with the full wo (no reduction needed).
Host gathers the 8 token shards.

All matmuls run as float32r (full PE rate at free-dim>=256, ~1.5e-4 rel err).
Softmax is computed in transposed [k, q] layout so that:
  - scoresT = kT.T @ qT comes straight from the projection layout,
  - exp runs on ACT with the 1/sqrt(128) scale and a global stability
    constant folded into the activation,
  - denominators come from ones-vector matmuls on the PE,
  - y^T = v.T @ attnT needs no transposes (v is transposed once in phase 1).
RoPE in [hd, tok] layout pairs adjacent partitions; the pair swap runs as a
PE matmul with a signed permutation matrix, the cos/sin scaling on DVE.
"""
import sys

sys.path.insert(0, "/opt/trn_rl_repo")

import numpy as np

B, S, D = 2, 2048, 4096
NH, NL, HD = 32, 8, 128
CORES = 8
QH = NH // CORES          # 4 q heads per core
TOK = B * S               # 4096
TPC = TOK // CORES        # 512 tokens per core (output sharding)
NT = 256                  # phase-1 token block width
KB_D = D // 128           # 32 contraction blocks over D
QT_W = 512                # phase-2 q tile width
N_QT = S // QT_W          # 4 q tiles per batch
N_KB = S // 128           # 16 k blocks per batch
WO_NT = 256               # phase-4 dout block width
SCALE = 1.0 / np.sqrt(np.float32(HD))

_CACHE = {}


def _build_nc(mode, c_sub, sim=False):
    """mode: 'causal' | 'full' | 'generic'. c_sub: global softmax shift.
    sim=True: single-core TimelineSim variant (collective replaced by DMAs)."""
    import concourse.bacc as bacc
    import concourse.mybir as mybir
    import concourse.tile as tile
    from contextlib import ExitStack

    F32 = mybir.dt.float32
    F32R = mybir.dt.float32r
    AT = mybir.ActivationFunctionType
    OP = mybir.AluOpType

    nc = bacc.Bacc("TRN2", target_bir_lowering=False, debug=False,
                   num_devices=1 if sim else CORES)

    xT_d = nc.dram_tensor("xT", (D, TOK), F32, kind="ExternalInput").ap()
    wqkvT_d = nc.dram_tensor("wqkvT", (D, (QH + 2) * HD), F32,
                             kind="ExternalInput").ap()
    woT_d = nc.dram_tensor("woT", (D, D), F32, kind="ExternalInput").ap()
    cosP_d = nc.dram_tensor("cosP", (HD, TOK), F32, kind="ExternalInput").ap()
    sinP_d = nc.dram_tensor("sinP", (HD, TOK), F32, kind="ExternalInput").ap()
    if mode == "generic":
        biasT_d = nc.dram_tensor("biasT", (S, S), F32, kind="ExternalInput").ap()
    out_d = nc.dram_tensor("out", (TPC, D), F32, kind="ExternalOutput").ap()

    ident_h = nc.inline_tensor(np.eye(128, dtype=np.float32), name="ident")
    pswap = np.zeros((128, 128), dtype=np.float32)
    for i in range(64):
        pswap[2 * i, 2 * i + 1] = -1.0
        pswap[2 * i + 1, 2 * i] = 1.0
    pswapT_h = nc.inline_tensor(np.ascontiguousarray(pswap.T), name="pswapT")
    tri = np.zeros((4, 128, QT_W), dtype=np.float32)
    for j in range(4):
        for p in range(128):
            tri[j, p, p + 128 * j:] = 1.0
    tri_h = nc.inline_tensor(tri, name="trimask")
    ones_col_h = nc.inline_tensor(np.ones((128, 1), np.float32), name="ones_col")
    ones_row_h = nc.inline_tensor(np.ones((1, 128), np.float32), name="ones_row")

    NROW = (QH + 2) * HD          # 768 qkv rows per core
    NM = NROW // 128              # 6 m tiles (0..3 q heads, 4 kT, 5 vT)
    NNT = TOK // NT               # 16 phase-1 token blocks
    NPAIR = KB_D // 2             # 16 kb pairs

    with tile.TileContext(nc) as tc, ExitStack() as glob:
        dram = glob.enter_context(tc.tile_pool(name="dram", bufs=1, space="DRAM"))
        consts = glob.enter_context(tc.tile_pool(name="consts", bufs=1))

        qkvT_t = dram.tile([NROW, TOK], F32R)
        v_t = dram.tile([TOK, HD], F32R)
        # split A2A: hp=0 carries head-locals {0,1}, hp=1 carries {2,3}
        a2a_in = [dram.tile([TOK // 2, TPC], F32, name=f"a2a_in{hp}")
                  for hp in range(2)]
        a2a_out = [dram.tile([TOK // 2, TPC], F32, name=f"a2a_out{hp}")
                   for hp in range(2)]

        ident_sb = consts.tile([128, 128], F32)
        nc.sync.dma_start(ident_sb[:], ident_h.ap())
        pswapT_sb = consts.tile([128, 128], F32R)
        nc.sync.dma_start(pswapT_sb[:], pswapT_h.ap().bitcast(F32R))
        ones_col_sb = consts.tile([128, 1], F32R)
        nc.sync.dma_start(ones_col_sb[:], ones_col_h.ap().bitcast(F32R))
        ones_row_sb = consts.tile([1, 128], F32R)
        nc.sync.dma_start(ones_row_sb[:], ones_row_h.ap().bitcast(F32R))
        tri_sb = consts.tile([128, 4 * QT_W], F32)
        for j in range(4):
            nc.sync.dma_start(tri_sb[:, QT_W * j:QT_W * (j + 1)], tri_h.ap()[j])

        # ================= phase 1: qkv projection + rope + v transpose
        with ExitStack() as ctx1:
            p1_w = ctx1.enter_context(tc.tile_pool(name="p1_w", bufs=1))
            p1_x = ctx1.enter_context(tc.tile_pool(name="p1_x", bufs=2))
            p1_cs = ctx1.enter_context(tc.tile_pool(name="p1_cs", bufs=2))
            p1_st = ctx1.enter_context(tc.tile_pool(name="p1_st", bufs=3))
            p1_so = ctx1.enter_context(tc.tile_pool(name="p1_so", bufs=2))
            p1_pa = ctx1.enter_context(tc.tile_pool(name="p1_pa", bufs=3, space="PSUM"))
            p1_pb = ctx1.enter_context(tc.tile_pool(name="p1_pb", bufs=2, space="PSUM"))
            p1_pt = ctx1.enter_context(tc.tile_pool(name="p1_pt", bufs=2, space="PSUM"))

            # kb-paired weight tiles; interleave with nt=0 x DMAs for startup
            w_sb = []
            x0_sb = []
            for i in range(NPAIR):
                wt = p1_w.tile([128, 2 * NROW], F32R, name=f"w{i}")
                nc.sync.dma_start(
                    wt[:].rearrange("p (two c) -> p two c", two=2),
                    wqkvT_d[256 * i:256 * (i + 1), :]
                    .rearrange("(two p) c -> p two c", p=128).bitcast(F32R))
                w_sb.append(wt)
                xt = p1_x.tile([128, 2 * NT], F32R, name=f"x{i}")
                nc.sync.dma_start(
                    xt[:].rearrange("p (two c) -> p two c", two=2),
                    xT_d[256 * i:256 * (i + 1), 0:NT]
                    .rearrange("(two p) c -> p two c", p=128).bitcast(F32R))
                x0_sb.append(xt)

            def wv(kb):       # lhsT view of wqkv kb-block
                return w_sb[kb // 2][:, NROW * (kb % 2):NROW * (kb % 2) + NROW]

            for nt in range(NNT):
                c0 = NT * nt
                if nt == 0:
                    x_sb = x0_sb
                else:
                    x_sb = []
                    for i in range(NPAIR):
                        xt = p1_x.tile([128, 2 * NT], F32R, name=f"x{i}")
                        nc.sync.dma_start(
                            xt[:].rearrange("p (two c) -> p two c", two=2),
                            xT_d[256 * i:256 * (i + 1), c0:c0 + NT]
                            .rearrange("(two p) c -> p two c", p=128).bitcast(F32R))
                        x_sb.append(xt)

                def xv(kb):
                    return x_sb[kb // 2][:, NT * (kb % 2):NT * (kb % 2) + NT]

                cos_sb = p1_cs.tile([128, NT], F32, name="cos_sb")
                nc.sync.dma_start(cos_sb[:], cosP_d[:, c0:c0 + NT])
                sin_sb = p1_cs.tile([128, NT], F32, name="sin_sb")
                nc.sync.dma_start(sin_sb[:], sinP_d[:, c0:c0 + NT])

                stage = p1_so.tile([128, (NM - 1) * NT], F32R, name="stage")
                for m in range(NM):
                    pa = p1_pa.tile([128, NT], F32, name="pa")
                    for kb in range(KB_D):
                        nc.tensor.matmul(pa[:], wv(kb)[:, 128 * m:128 * (m + 1)],
                                         xv(kb),
                                         start=(kb == 0), stop=(kb == KB_D - 1))
                    if m < NM - 1:
                        # rope for q heads + k head; write into packed stage
                        a_sb = p1_st.tile([128, NT], F32R, name="a_sb")
                        nc.vector.tensor_copy(a_sb[:], pa[:])
                        pb = p1_pb.tile([128, NT], F32, name="pb")
                        nc.tensor.matmul(pb[:], pswapT_sb[:], a_sb[:],
                                         start=True, stop=True)
                        tcos = p1_st.tile([128, NT], F32, name="tcos")
                        nc.vector.tensor_tensor(tcos[:], pa[:], cos_sb[:], op=OP.mult)
                        tsin = p1_st.tile([128, NT], F32, name="tsin")
                        nc.vector.tensor_tensor(tsin[:], pb[:], sin_sb[:], op=OP.mult)
                        nc.vector.tensor_tensor(stage[:, NT * m:NT * (m + 1)],
                                                tcos[:], tsin[:], op=OP.add)
                    else:
                        # vT -> v natural via PE transpose, packed write-out
                        vst = p1_st.tile([128, NT], F32, name="vst")
                        nc.scalar.copy(vst[:], pa[:])
                        vpack = p1_st.tile([128, NT], F32R, name="vpack")
                        for j in range(NT // 128):
                            pt = p1_pt.tile([128, 128], F32, name="pt")
                            nc.tensor.transpose(
                                pt[:], vst[:, 128 * j:128 * (j + 1)], ident_sb[:])
                            nc.vector.tensor_copy(
                                vpack[:, 128 * j:128 * (j + 1)], pt[:])
                        nc.sync.dma_start(
                            v_t[c0:c0 + NT, :]
                            .rearrange("(j p) d -> p j d", p=128),
                            vpack[:].rearrange("p (j d) -> p j d", d=128))
                # one packed DMA for the 5 roped m-tiles
                nc.sync.dma_start(
                    qkvT_t[0:(NM - 1) * 128, c0:c0 + NT]
                    .rearrange("(m p) c -> p m c", p=128),
                    stage[:].rearrange("p (m c) -> p m c", c=NT))

        # ================= phase 4 woT pool opened early for prefetch
        p4_w = glob.enter_context(tc.tile_pool(name="p4_w", bufs=2))

        # ================= phase 2: attention, head-pair outer for split A2A
        with ExitStack() as ctx2:
            p2_qk = ctx2.enter_context(tc.tile_pool(name="p2_qk", bufs=2))
            p2_v = ctx2.enter_context(tc.tile_pool(name="p2_v", bufs=2))
            p2_at = ctx2.enter_context(tc.tile_pool(name="p2_at", bufs=4))
            p2_ms = ctx2.enter_context(tc.tile_pool(name="p2_ms", bufs=3))
            p2_psc = ctx2.enter_context(tc.tile_pool(name="p2_psc", bufs=3, space="PSUM"))
            p2_py = ctx2.enter_context(tc.tile_pool(name="p2_py", bufs=2, space="PSUM"))
            p2_ps = ctx2.enter_context(tc.tile_pool(name="p2_ps", bufs=2, space="PSUM"))
            p2_pr = ctx2.enter_context(tc.tile_pool(name="p2_pr", bufs=1, space="PSUM"))
            if mode == "generic":
                p2_bias = ctx2.enter_context(tc.tile_pool(name="p2_bias", bufs=4))

            for hp in range(2):
                for b in range(B):
                    t0 = S * b
                    kT = p2_qk.tile([128, S], F32R, name="kT")
                    nc.sync.dma_start(kT[:],
                                      qkvT_t[QH * 128:(QH + 1) * 128, t0:t0 + S])
                    v_sb = []
                    for i in range(N_KB // 4):
                        vt = p2_v.tile([128, 4 * HD], F32R, name=f"v{i}")
                        nc.sync.dma_start(
                            vt[:].rearrange("p (j d) -> p j d", d=HD),
                            v_t[t0 + 512 * i:t0 + 512 * (i + 1), :]
                            .rearrange("(j p) d -> p j d", p=128))
                        v_sb.append(vt)

                    def vvw(kb):
                        return v_sb[kb // 4][:, HD * (kb % 4):HD * (kb % 4) + HD]

                    for h in (2 * hp, 2 * hp + 1):
                        qT = p2_qk.tile([128, S], F32R, name="qT")
                        nc.sync.dma_start(
                            qT[:], qkvT_t[128 * h:128 * (h + 1), t0:t0 + S])
                        for qt in range(N_QT):
                            kb_max = 4 * qt + 4 if mode == "causal" else N_KB
                            py = p2_py.tile([128, QT_W], F32, name="py")
                            s_t = p2_ps.tile([1, QT_W], F32, name="s_t")
                            ps_s = s_t
                            for kb in range(kb_max):
                                # diag tiles: only columns >= c0 are unmasked;
                                # narrowed ops never touch [0:c0)
                                diag_j = kb - 4 * qt if (
                                    mode == "causal" and kb >= 4 * qt) else -1
                                c0 = 128 * diag_j if diag_j > 0 else 0
                                w = QT_W - c0
                                psc = p2_psc.tile([128, QT_W], F32, name="psc")
                                nc.tensor.matmul(
                                    psc[:, c0:QT_W], kT[:, 128 * kb:128 * (kb + 1)],
                                    qT[:, QT_W * qt + c0:QT_W * (qt + 1)],
                                    start=True, stop=True)
                                if mode == "generic":
                                    bt = p2_bias.tile([128, QT_W], F32, name="bt")
                                    nc.sync.dma_start(
                                        bt[:], biasT_d[128 * kb:128 * (kb + 1),
                                                       QT_W * qt:QT_W * (qt + 1)])
                                    nc.vector.tensor_tensor(psc[:], psc[:], bt[:],
                                                            op=OP.add)
                                at = p2_at.tile([128, QT_W], F32R, name="at")
                                nc.scalar.activation(at[:, c0:QT_W], psc[:, c0:QT_W],
                                                     AT.Exp,
                                                     bias=-float(c_sub),
                                                     scale=float(SCALE))
                                if diag_j >= 0:
                                    nc.vector.tensor_tensor(
                                        at[:, c0:c0 + 128],
                                        at[:, c0:c0 + 128].bitcast(F32),
                                        tri_sb[:, 0:128], op=OP.mult)
                                nc.tensor.matmul(py[:, c0:QT_W], vvw(kb),
                                                 at[:, c0:QT_W],
                                                 start=(kb == 0),
                                                 stop=(kb == kb_max - 1),
                                                 skip_group_check=True)
                                nc.tensor.matmul(ps_s[:, c0:QT_W], ones_col_sb[:],
                                                 at[:, c0:QT_W],
                                                 start=(kb == 0),
                                                 stop=(kb == kb_max - 1),
                                                 skip_group_check=True)
                            # normalize: yT = py * (1/sums) replicated over parts
                            recip = p2_ms.tile([1, QT_W], F32R, name="recip")
                            with nc.allow_low_precision(reason="f32r recip"):
                                nc.vector.reciprocal(recip[:], ps_s)
                            pr = p2_pr.tile([128, QT_W], F32, name="pr")
                            nc.tensor.matmul(pr[:], ones_row_sb[:], recip[:],
                                             start=True, stop=True)
                            rep_sb = p2_ms.tile([128, QT_W], F32, name="rep_sb")
                            nc.vector.tensor_copy(rep_sb[:], pr[:])
                            yT_sb = p2_ms.tile([128, QT_W], F32, name="yT_sb")
                            nc.vector.tensor_tensor(yT_sb[:], py[:], rep_sb[:],
                                                    op=OP.mult)
                            # A2A input: chunk j = 4b + qt, rows 128*(h%2)
                            j = 4 * b + qt
                            r0 = (TPC // 2) * j + 128 * (h % 2)
                            nc.sync.dma_start(
                                a2a_in[hp][r0:r0 + 128, :], yT_sb[:])
                # -------- A2A for this head-pair (overlaps later compute)
                if sim:
                    for j in range(CORES):
                        nc.sync.dma_start(
                            a2a_out[hp][(TPC // 2) * j:(TPC // 2) * (j + 1), :],
                            a2a_in[hp][(TPC // 2) * j:(TPC // 2) * (j + 1), :])
                else:
                    nc.gpsimd.collective_compute(
                        "AllToAll", mybir.AluOpType.bypass,
                        replica_groups=[list(range(CORES))],
                        ins=[a2a_in[hp][:]], outs=[a2a_out[hp][:]],
                    )

        # ================= phase 4: output projection out = y @ wo.T
        # yT_full row for global kb: i=kb//4, hl=kb%4 -> a2a_out[hl//2]
        # rows 256*i + 128*(hl%2). Emit hp=0 kbs first so phase 4 starts
        # while A2A #1 is the only one finished.
        kb_order = [kb for kb in range(KB_D) if (kb % 4) < 2] + \
                   [kb for kb in range(KB_D) if (kb % 4) >= 2]
        with ExitStack() as ctx4:
            p4_y = ctx4.enter_context(tc.tile_pool(name="p4_y", bufs=1))
            p4_st = ctx4.enter_context(tc.tile_pool(name="p4_st", bufs=4))
            p4_po = ctx4.enter_context(tc.tile_pool(name="p4_po", bufs=4, space="PSUM"))

            y_sb = {}
            for kb in kb_order:
                i, hl = kb // 4, kb % 4
                r0 = 256 * i + 128 * (hl % 2)
                yt = p4_y.tile([128, TPC], F32R, name=f"y{kb}")
                nc.sync.dma_start(
                    yt[:], a2a_out[hl // 2][r0:r0 + 128, :].bitcast(F32R))
                y_sb[kb] = yt
            for do in range(D // WO_NT):
                wo_sb = []
                for i in range(NPAIR):
                    wt = p4_w.tile([128, 2 * WO_NT], F32R, name=f"wo{i}")
                    nc.sync.dma_start(
                        wt[:].rearrange("p (two c) -> p two c", two=2),
                        woT_d[256 * i:256 * (i + 1),
                              WO_NT * do:WO_NT * (do + 1)]
                        .rearrange("(two p) c -> p two c", p=128).bitcast(F32R))
                    wo_sb.append(wt)

                def wov(kb):
                    return wo_sb[kb // 2][:, WO_NT * (kb % 2):WO_NT * (kb % 2 + 1)]

                for tb in range(TPC // 128):
                    po = p4_po.tile([128, WO_NT], F32, name="po")
                    for n, kb in enumerate(kb_order):
                        nc.tensor.matmul(po[:], y_sb[kb][:, 128 * tb:128 * (tb + 1)],
                                         wov(kb),
                                         start=(n == 0), stop=(n == KB_D - 1))
                    o_sb = p4_st.tile([128, WO_NT], F32, name="o_sb")
                    nc.scalar.copy(o_sb[:], po[:])
                    nc.sync.dma_start(
                        out_d[128 * tb:128 * (tb + 1), WO_NT * do:WO_NT * (do + 1)],
                        o_sb[:])

    nc.compile()
    return nc


def _prepare(x, freqs_cis, mask, wqkv_w, wo_w):
    """Host-side prep: mode detection, stability constant, input maps."""
    x = np.asarray(x, dtype=np.float32)
    freqs_cis = np.asarray(freqs_cis, dtype=np.float32)
    mask = np.asarray(mask)
    wqkv_w = np.asarray(wqkv_w, dtype=np.float32)
    wo_w = np.asarray(wo_w, dtype=np.float32)

    m2 = mask.reshape(mask.shape[-2], mask.shape[-1])
    if np.array_equal(m2, np.tril(np.ones((S, S), dtype=bool))):
        mode = "causal"
    elif m2.all():
        mode = "full"
    else:
        mode = "generic"

    x2 = x.reshape(TOK, D)
    xT = np.ascontiguousarray(x2.T)
    woT = np.ascontiguousarray(wo_w.T)

    cos = freqs_cis[:, :, 0].T          # [64, S]
    sin = freqs_cis[:, :, 1].T
    cosP = np.repeat(cos, 2, axis=0)    # [128, S]
    sinP = np.repeat(sin, 2, axis=0)
    cosP = np.ascontiguousarray(np.tile(cosP, (1, B)))  # [128, TOK]
    sinP = np.ascontiguousarray(np.tile(sinP, (1, B)))

    # softmax stability probe: rope'd scores for head 0, batch 0, 128 q rows
    wq0 = wqkv_w[:HD]                   # [128, D]
    wk0 = wqkv_w[NH * HD:NH * HD + HD]  # [128, D]
    qs = x2[:128] @ wq0.T               # [128, 128]
    ks = x2[:S] @ wk0.T                 # [S, 128]

    def rope_np(t, fc):
        ts = t.reshape(t.shape[0], HD // 2, 2)
        c, s_ = fc[:t.shape[0], :, 0], fc[:t.shape[0], :, 1]
        out = np.empty_like(ts)
        out[:, :, 0] = ts[:, :, 0] * c - ts[:, :, 1] * s_
        out[:, :, 1] = ts[:, :, 1] * c + ts[:, :, 0] * s_
        return out.reshape(t.shape)

    qs = rope_np(qs, freqs_cis)
    ks = rope_np(ks, freqs_cis)
    smax = float(np.max(np.abs(qs @ ks.T)) * SCALE)
    c_sub = 0.0 if smax < 25.0 else smax + 5.0

    in_maps = []
    for c in range(CORES):
        wq_c = wqkv_w[QH * HD * c:QH * HD * (c + 1)]
        wk_c = wqkv_w[NH * HD + HD * c:NH * HD + HD * (c + 1)]
        wv_c = wqkv_w[(NH + NL) * HD + HD * c:(NH + NL) * HD + HD * (c + 1)]
        wqkvT_c = np.ascontiguousarray(np.vstack([wq_c, wk_c, wv_c]).T)
        m = {"xT": xT, "wqkvT": wqkvT_c, "woT": woT, "cosP": cosP, "sinP": sinP}
        if mode == "generic":
            m["biasT"] = np.ascontiguousarray(
                np.where(m2.T, np.float32(0), np.float32(-1e30)))
        in_maps.append(m)
    return mode, c_sub, in_maps


def _get_nc(mode, c_sub):
    key = (mode, round(float(c_sub), 3))
    if key not in _CACHE:
        _CACHE[key] = _build_nc(mode, c_sub)
    return _CACHE[key]


def kernel(x, freqs_cis, mask, wqkv_w, wo_w):
    from concourse import bass_utils
    mode, c_sub, in_maps = _prepare(x, freqs_cis, mask, wqkv_w, wo_w)
    nc = _get_nc(mode, c_sub)
    res = bass_utils.run_bass_kernel_spmd(nc, in_maps, core_ids=list(range(CORES)))
    out = np.concatenate([res.results[c]["out"] for c in range(CORES)], axis=0)
    return out.reshape(B, S, D)
